# revision 4
# baseline (speedup 1.0000x reference)
"""Trainium2 Bass kernel for a dense transformer block (B=4, T=2048, C=1024, H=16).

Sharding: data-parallel over tokens. Core i owns batch b=i//2, token-half i%2
(1024 tokens). Each core redundantly computes LN1/K/V for its batch's full 2048
tokens (+12% PE) so there are no collectives at all.

Everything on-chip is feature-major ([C, tok]); the host pre-transposes x and
post-transposes the output, so the kernel needs no on-chip transposes:
  - LN stats via ones-matrix matmuls on PE (partition reduction),
  - QKV/proj/MLP matmuls contract C on the partition axis,
  - attention scores computed as scores^T [keys, queries] so softmax's exp is a
    single ACT pass per PSUM tile and the denominator rides the AV matmul as a
    65th all-ones column of V,
  - per-head q is stored twice with the other head's rows zeroed, so score
    matmuls contract K=128 (mixing K=64 matmuls at partition bases 0 and 64 in
    one program faults the runtime).
Matmuls run in bf16 (f32 PSUM accumulation); the residual stream stays f32.
LN scale (g) is folded into weight rows and LN shift (beta) into biases on the
host, so on-chip LN is a pure normalize.
"""

import sys

if "/opt/trn_rl_repo" not in sys.path:
    sys.path.insert(0, "/opt/trn_rl_repo")

import numpy as np
import ml_dtypes

B, T, C, H, HD = 4, 2048, 1024, 16, 64
FF = 4 * C
TO = T // 2          # tokens owned per core
NC_CHUNKS = C // 128  # 8
NF_CHUNKS = FF // 128  # 32
EPS = 1e-5
SCALE = C ** -0.5     # 1/32
BF16 = ml_dtypes.bfloat16

_BUILT = None


def _emit(nc, tc, aps, has_bv):
    import concourse.bass as bass
    from concourse import mybir
    from concourse.bass import ts
    F32 = mybir.dt.float32
    BF = mybir.dt.bfloat16
    AF = mybir.ActivationFunctionType
    ADD = mybir.AluOpType.add
    from contextlib import ExitStack

    xT, wq, wk, wv, wproj, w1, w2, bq, bk, bp, b2c, b1c, bvrow, x2d, outT = (
        aps["xT"], aps["wq"], aps["wk"], aps["wv"], aps["wproj"], aps["w1"],
        aps["w2"], aps["bq"], aps["bk"], aps["bp"], aps["b2c"], aps["b1c"],
        aps.get("bvrow"), aps["x2d"], aps["outT"])

    ctx = ExitStack()
    with ctx:
        const = ctx.enter_context(tc.tile_pool(name="const", bufs=1))
        misc = ctx.enter_context(tc.tile_pool(name="misc", bufs=3))
        wpool = ctx.enter_context(tc.tile_pool(name="wpool", bufs=8))
        psum = ctx.enter_context(tc.tile_pool(name="psum", bufs=3, space="PSUM"))

        def ps_sc():
            return psum.tile([128, 2, 512], F32, name="ps_sc", tag="sc", bufs=3)

        def ps_mm():
            return psum.tile([128, 512], F32, name="ps_mm", tag="mm", bufs=2)

        # constants / biases
        ones_sc = const.tile([128, 128], BF, name="ones_sc")
        nc.vector.memset(ones_sc, 1.0 / C)
        eps_sb = const.tile([128, 1], F32, name="eps_sb")
        nc.vector.memset(eps_sb, EPS)
        bq_sb = const.tile([128, 8], F32, name="bq_sb")
        bk_sb = const.tile([128, 8], F32, name="bk_sb")
        bp_sb = const.tile([128, 8], F32, name="bp_sb")
        b2_sb = const.tile([128, 8], F32, name="b2_sb")
        b1_sb = const.tile([128, 32], F32, name="b1_sb")
        nc.sync.dma_start(out=bq_sb, in_=bq)
        nc.sync.dma_start(out=bk_sb, in_=bk)
        nc.sync.dma_start(out=bp_sb, in_=bp)
        nc.sync.dma_start(out=b2_sb, in_=b2c)
        nc.sync.dma_start(out=b1_sb, in_=b1c)
        if has_bv:
            bv_sb = const.tile([1, 1024], F32, name="bv_sb")
            nc.sync.dma_start(out=bv_sb, in_=bvrow)
            bvb = const.tile([128, 1024], F32, name="bvb")
            nc.gpsimd.partition_broadcast(bvb, bv_sb)

        def layernorm_feature_major(src_dram, n_tc, hbuf_pool, xbf_pool, tc_off=0):
            """Emit LN over token-chunks of 512; yields per-tc bf16 h tiles.

            src_dram: [8, 128, ntok] DRAM ap. Returns list of h tiles (one per
            tc, [128, 8, 512] bf16), emitted lazily via generator.
            """
            for tci in range(n_tc):
                tcg = tci + tc_off
                xbf = xbf_pool.tile([128, 8, 512], BF, name="xbf", tag="xbf")
                st = ps_sc()  # [:,0,:] = mean, [:,1,:] = E[x^2] (replicated rows)
                for c in range(NC_CHUNKS):
                    xs = misc.tile([128, 512], F32, name="xs", tag="xs", bufs=3)
                    nc.sync.dma_start(out=xs, in_=src_dram[c, :, ts(tcg, 512)])
                    nc.scalar.copy(out=xbf[:, c, :], in_=xs)
                    xsq = misc.tile([128, 512], BF, name="xsq", tag="xsq", bufs=3)
                    nc.vector.tensor_mul(out=xsq, in0=xs, in1=xs)
                    nc.tensor.matmul(st[:, 0, :], ones_sc, xbf[:, c, :],
                                     start=(c == 0), stop=(c == NC_CHUNKS - 1),
                                     skip_group_check=True)
                    nc.tensor.matmul(st[:, 1, :], ones_sc, xsq,
                                     start=(c == 0), stop=(c == NC_CHUNKS - 1),
                                     skip_group_check=True)
                musq = misc.tile([128, 512], F32, name="musq", tag="stat", bufs=3)
                nc.scalar.square(out=musq, in_=st[:, 0, :])
                var = misc.tile([128, 512], F32, name="var", tag="stat", bufs=3)
                nc.vector.tensor_sub(out=var, in0=st[:, 1, :], in1=musq)
                sd = misc.tile([128, 512], F32, name="sd", tag="stat", bufs=3)
                nc.scalar.activation(out=sd, in_=var, func=AF.Sqrt, bias=eps_sb)
                s_t = misc.tile([128, 512], F32, name="s_t", tag="stat", bufs=3)
                nc.vector.reciprocal(out=s_t, in_=sd)
                h_t = hbuf_pool.tile([128, 8, 512], BF, name="h_t", tag="h")
                for c in range(NC_CHUNKS):
                    d = misc.tile([128, 512], F32, name="d", tag="xs", bufs=3)
                    nc.vector.tensor_sub(out=d, in0=xbf[:, c, :], in1=st[:, 0, :])
                    nc.vector.tensor_mul(out=h_t[:, c, :], in0=d, in1=s_t)
                yield tci, h_t

        # ============ scope A: LN1 + QKV + attention + proj ============
        with tc.tile_pool(name="attn", bufs=1) as attn, \
             tc.tile_pool(name="expp", bufs=3) as expp, \
             tc.tile_pool(name="stage", bufs=2) as stage:
            k_sb = attn.tile([128, 8, T], BF, name="k_sb")
            qz0 = attn.tile([128, 8, TO], BF, name="qz0")
            qz1 = attn.tile([128, 8, TO], BF, name="qz1")
            v_aug = attn.tile([128, 16, 16, 65], BF, name="v_aug")
            o_sb = attn.tile([128, 8, TO], BF, name="o_sb")
            nc.vector.memset(qz0[64:128, :, :], 0.0)
            nc.vector.memset(qz1[0:64, :, :], 0.0)
            nc.vector.memset(v_aug[:, :, :, 64:65], 1.0)

            with tc.tile_pool(name="lnp", bufs=2) as lnp, \
                 tc.tile_pool(name="xbfp", bufs=1) as xbfp:
                for tci, h_t in layernorm_feature_major(xT, 4, lnp, xbfp):
                    # K projection for this token chunk
                    wt = [wpool.tile([128, 1024], BF, name="wt", tag="w")
                          for _ in range(NC_CHUNKS)]
                    for c in range(NC_CHUNKS):
                        nc.sync.dma_start(out=wt[c], in_=wk[c])
                    for m in range(NC_CHUNKS):
                        kp = ps_mm()
                        for c in range(NC_CHUNKS):
                            nc.tensor.matmul(kp, wt[c][:, ts(m, 128)], h_t[:, c, :],
                                             start=(c == 0), stop=(c == NC_CHUNKS - 1))
                        nc.scalar.activation(out=k_sb[:, m, ts(tci, 512)], in_=kp,
                                             func=AF.Identity, bias=bk_sb[:, m:m + 1])
                    # V projection (token-major out) for this chunk
                    wtv = [wpool.tile([128, 1024], BF, name="wtv", tag="w")
                           for _ in range(NC_CHUNKS)]
                    for c in range(NC_CHUNKS):
                        nc.sync.dma_start(out=wtv[c], in_=wv[c])
                    for nch in range(2):
                        for tt in range(4):
                            vp = ps_mm()
                            for c in range(NC_CHUNKS):
                                nc.tensor.matmul(vp, h_t[:, c, ts(tt, 128)],
                                                 wtv[c][:, ts(nch, 512)],
                                                 start=(c == 0),
                                                 stop=(c == NC_CHUNKS - 1))
                            dst = v_aug[:, tci * 4 + tt, nch * 8:(nch + 1) * 8, 0:64]
                            src = vp.rearrange("p (h d) -> p h d", h=8)
                            if has_bv:
                                bslice = bvb[:, ts(nch, 512)].rearrange(
                                    "p (h d) -> p h d", h=8)
                                nc.vector.tensor_add(out=dst, in0=src, in1=bslice)
                            else:
                                nc.vector.tensor_copy(out=dst, in_=src)
                    # Q projection (own tokens only, split into zero-masked pair)
                    if tci < 2:
                        wtq = [wpool.tile([128, 1024], BF, name="wtq", tag="w")
                               for _ in range(NC_CHUNKS)]
                        for c in range(NC_CHUNKS):
                            nc.sync.dma_start(out=wtq[c], in_=wq[c])
                        for m in range(NC_CHUNKS):
                            qp = ps_mm()
                            for c in range(NC_CHUNKS):
                                nc.tensor.matmul(qp, wtq[c][:, ts(m, 128)],
                                                 h_t[:, c, :], start=(c == 0),
                                                 stop=(c == NC_CHUNKS - 1))
                            nc.scalar.activation(
                                out=qz0[0:64, m, ts(tci, 512)], in_=qp[0:64, :],
                                func=AF.Identity, bias=bq_sb[0:64, m:m + 1])
                            nc.scalar.activation(
                                out=qz1[64:128, m, ts(tci, 512)], in_=qp[64:128, :],
                                func=AF.Identity, bias=bq_sb[64:128, m:m + 1])

            # ---- attention + proj, per query chunk ----
            for qc in range(2):
                for h in range(H):
                    hp = h // 2
                    qz = qz0 if h % 2 == 0 else qz1
                    avp = ps_mm()
                    for g in range(8):
                        scp = ps_sc()
                        for j in range(2):
                            sk = g * 2 + j
                            nc.tensor.matmul(scp[:, j, :], k_sb[:, hp, ts(sk, 128)],
                                             qz[:, hp, ts(qc, 512)],
                                             start=True, stop=True)
                        et = expp.tile([128, 2, 512], BF, name="et", tag="et")
                        nc.scalar.activation(out=et, in_=scp, func=AF.Exp,
                                             scale=SCALE)
                        for j in range(2):
                            nc.tensor.matmul(avp[0:65, :],
                                             v_aug[:, g * 2 + j, h, :], et[:, j, :],
                                             start=(g == 0 and j == 0),
                                             stop=(g == 7 and j == 1),
                                             skip_group_check=True)
                    r_t = misc.tile([1, 512], F32, name="r_t", tag="r", bufs=3)
                    nc.vector.reciprocal(out=r_t, in_=avp[64:65, :])
                    rb_t = misc.tile([64, 512], F32, name="rb_t", tag="rb", bufs=3)
                    nc.gpsimd.partition_broadcast(rb_t, r_t)
                    p0 = (h % 2) * 64
                    nc.vector.tensor_mul(out=o_sb[p0:p0 + 64, hp, ts(qc, 512)],
                                         in0=avp[0:64, :], in1=rb_t)
                # proj + residual -> x2 (DRAM)
                wtp = [wpool.tile([128, 1024], BF, name="wtp", tag="w")
                       for _ in range(NC_CHUNKS)]
                for c in range(NC_CHUNKS):
                    nc.sync.dma_start(out=wtp[c], in_=wproj[c])
                for m in range(NC_CHUNKS):
                    pp = ps_mm()
                    for c in range(NC_CHUNKS):
                        nc.tensor.matmul(pp, wtp[c][:, ts(m, 128)],
                                         o_sb[:, c, ts(qc, 512)],
                                         start=(c == 0), stop=(c == NC_CHUNKS - 1))
                    xres = stage.tile([128, 512], F32, name="xres", tag="xres")
                    nc.sync.dma_start(out=xres, in_=xT[m, :, ts(qc, 512)])
                    x2t = stage.tile([128, 512], F32, name="x2t", tag="x2t")
                    nc.vector.scalar_tensor_tensor(out=x2t, in0=pp,
                                                   scalar=bp_sb[:, m:m + 1],
                                                   in1=xres, op0=ADD, op1=ADD)
                    nc.sync.dma_start(out=x2d[m, :, ts(qc, 512)], in_=x2t)

        # ============ scope B: LN2 + MLP ============
        with tc.tile_pool(name="mlp", bufs=1) as mlp, \
             tc.tile_pool(name="lnp2", bufs=2) as lnp2, \
             tc.tile_pool(name="xbfp2", bufs=1) as xbfp2, \
             tc.tile_pool(name="outp", bufs=3) as outp:
            for qc in range(2):
                z_sb = mlp.tile([128, 32, 512], BF, name="z_sb", tag="z")
                for _tci, h2 in layernorm_feature_major(x2d, 1, lnp2, xbfp2,
                                                        tc_off=qc):
                    pass
                # MLP1: z = relu(W1' @ h2 + b1'), 8 psum accumulators per m-group
                for mg in range(4):
                    accs = [ps_sc() for _ in range(3)] + [ps_mm() for _ in range(2)]

                    def acc(i):
                        return accs[i // 2][:, i % 2, :] if i < 6 else accs[3 + (i - 6)]

                    for k in range(NC_CHUNKS):
                        w1t = wpool.tile([128, 1024], BF, name="w1t", tag="w")
                        nc.sync.dma_start(out=w1t, in_=w1[mg, k])
                        for m8 in range(8):
                            nc.tensor.matmul(acc(m8), w1t[:, ts(m8, 128)],
                                             h2[:, k, :], start=(k == 0),
                                             stop=(k == NC_CHUNKS - 1),
                                             skip_group_check=True)
                    for m8 in range(8):
                        m = mg * 8 + m8
                        nc.scalar.activation(out=z_sb[:, m, :], in_=acc(m8),
                                             func=AF.Relu, bias=b1_sb[:, m:m + 1])
                # MLP2: out = W2 @ z + b2 + x2
                accs = [ps_sc() for _ in range(3)] + [ps_mm() for _ in range(2)]

                def acc2(i):
                    return accs[i // 2][:, i % 2, :] if i < 6 else accs[3 + (i - 6)]

                for k in range(NF_CHUNKS):
                    w2t = wpool.tile([128, 1024], BF, name="w2t", tag="w")
                    nc.sync.dma_start(out=w2t, in_=w2[k])
                    for m in range(8):
                        nc.tensor.matmul(acc2(m), w2t[:, ts(m, 128)], z_sb[:, k, :],
                                         start=(k == 0), stop=(k == NF_CHUNKS - 1),
                                         skip_group_check=True)
                for m in range(8):
                    xr2 = outp.tile([128, 512], F32, name="xr2", tag="xr2")
                    nc.sync.dma_start(out=xr2, in_=x2d[m, :, ts(qc, 512)])
                    ot = outp.tile([128, 512], F32, name="ot", tag="ot")
                    nc.vector.scalar_tensor_tensor(out=ot, in0=acc2(m),
                                                   scalar=b2_sb[:, m:m + 1],
                                                   in1=xr2, op0=ADD, op1=ADD)
                    nc.sync.dma_start(out=outT[m, :, ts(qc, 512)], in_=ot)


def _build(has_bv):
    import concourse.bass as bass
    from concourse import bacc, mybir, tile
    F32 = mybir.dt.float32
    BF = mybir.dt.bfloat16

    nc = bacc.Bacc("TRN2", target_bir_lowering=False, debug=False,
                   enable_asserts=False, num_devices=8)
    aps = {}
    aps["xT"] = nc.dram_tensor("xT", [8, 128, T], F32, kind="ExternalInput").ap()
    for n in ("wq", "wk", "wv", "wproj"):
        aps[n] = nc.dram_tensor(n, [8, 128, 1024], BF, kind="ExternalInput").ap()
    aps["w1"] = nc.dram_tensor("w1", [4, 8, 128, 1024], BF, kind="ExternalInput").ap()
    aps["w2"] = nc.dram_tensor("w2", [32, 128, 1024], BF, kind="ExternalInput").ap()
    for n in ("bq", "bk", "bp", "b2c"):
        aps[n] = nc.dram_tensor(n, [128, 8], F32, kind="ExternalInput").ap()
    aps["b1c"] = nc.dram_tensor("b1c", [128, 32], F32, kind="ExternalInput").ap()
    if has_bv:
        aps["bvrow"] = nc.dram_tensor("bvrow", [1, 1024], F32,
                                      kind="ExternalInput").ap()
    aps["x2d"] = nc.dram_tensor("x2d", [8, 128, TO], F32).ap()
    aps["outT"] = nc.dram_tensor("outT", [8, 128, TO], F32,
                                 kind="ExternalOutput").ap()

    with tile.TileContext(nc) as tcx:
        _emit(nc, tcx, aps, has_bv)
    nc.compile()
    return nc


def _prep_inputs(x, Wq, Wk, Wv, Wproj, bproj, W1, b1, W2, b2, g1, be1, g2, be2):
    """Host-side prep: fold LN affine into weights/biases, cast, lay out."""
    x = np.asarray(x, np.float32)
    g1 = np.asarray(g1, np.float32)
    be1 = np.asarray(be1, np.float32)
    g2 = np.asarray(g2, np.float32)
    be2 = np.asarray(be2, np.float32)

    def to2d(w):  # (H, C, hd) -> (C, H*hd)
        return np.asarray(w, np.float32).transpose(1, 0, 2).reshape(C, C)

    wq2, wk2, wv2 = to2d(Wq), to2d(Wk), to2d(Wv)
    Wproj = np.asarray(Wproj, np.float32)
    W1 = np.asarray(W1, np.float32)
    W2 = np.asarray(W2, np.float32)

    def fold(w, g):
        return (g[:, None] * w)

    wq_e, wk_e, wv_e = fold(wq2, g1), fold(wk2, g1), fold(wv2, g1)
    w1_e = fold(W1, g2)
    bias_q = be1 @ wq2
    bias_k = be1 @ wk2
    bias_v = be1 @ wv2
    bias_1 = np.asarray(b1, np.float32) + be2 @ W1
    bias_p = np.asarray(bproj, np.float32)
    bias_2 = np.asarray(b2, np.float32)

    def wchunks(w):  # (C, N) -> (8, 128, N) bf16 contiguous
        return np.ascontiguousarray(w.reshape(NC_CHUNKS, 128, -1).astype(BF16))

    wq_c, wk_c, wv_c, wp_c = (wchunks(wq_e), wchunks(wk_e), wchunks(wv_e),
                              wchunks(Wproj))
    w1_c = np.ascontiguousarray(
        w1_e.reshape(NC_CHUNKS, 128, 4, 1024).transpose(2, 0, 1, 3).astype(BF16))
    w2_c = np.ascontiguousarray(W2.reshape(NF_CHUNKS, 128, C).astype(BF16))

    def bvec(v):  # (N,) -> (128, N//128) partition-major
        return np.ascontiguousarray(
            np.asarray(v, np.float32).reshape(-1, 128).T)

    shared = {
        "wq": wq_c, "wk": wk_c, "wv": wv_c, "wproj": wp_c,
        "w1": w1_c, "w2": w2_c,
        "bq": bvec(bias_q), "bk": bvec(bias_k), "bp": bvec(bias_p),
        "b2c": bvec(bias_2), "b1c": bvec(bias_1),
    }
    has_bv = bool(np.any(bias_v != 0.0))
    if has_bv:
        shared["bvrow"] = np.ascontiguousarray(bias_v.reshape(1, C))

    in_maps = []
    for core in range(8):
        b, half = core // 2, core % 2
        xt = x[b].T  # (C, T)
        own = xt[:, half * TO:(half + 1) * TO]
        oth = xt[:, (1 - half) * TO:(2 - half) * TO]
        xcat = np.ascontiguousarray(
            np.concatenate([own, oth], axis=1).reshape(NC_CHUNKS, 128, T))
        m = dict(shared)
        m["xT"] = xcat
        in_maps.append(m)
    return in_maps, has_bv


def kernel(x, Wq, Wk, Wv, Wproj, bproj, W1, b1, W2, b2, g1, be1, g2, be2):
    global _BUILT
    from concourse.bass_utils import run_bass_kernel_spmd

    in_maps, has_bv = _prep_inputs(x, Wq, Wk, Wv, Wproj, bproj, W1, b1, W2, b2,
                                   g1, be1, g2, be2)
    if _BUILT is None or _BUILT[1] != has_bv:
        _BUILT = (_build(has_bv), has_bv)
    nc = _BUILT[0]
    res = run_bass_kernel_spmd(nc, in_maps, core_ids=list(range(8)))
    out = np.empty((B, T, C), np.float32)
    for core in range(8):
        b, half = core // 2, core % 2
        o = res.results[core]["outT"].reshape(C, TO)  # (feature, token)
        out[b, half * TO:(half + 1) * TO, :] = o.T
    return out
